# revision 6
# baseline (speedup 1.0000x reference)
"""Trainium2 Bass kernel for GRU regressor (B=256, T=512, F=64, H=512).

Data-parallel: batch sharded 32/core across 8 NeuronCores. Per core the
recurrence runs in a gate-major transposed layout: state h kept as
[128 partitions, 4 k-chunks x 32 batch] (hidden unit u = k*128+p), so the
per-step W_hh matmul is 48 [128,128] bf16 stationary chunks (FWL) with the
state as the moving operand, and all elementwise work runs on [128, small]
tiles. Input projections (x @ W_ih.T + biases) are precomputed in 16-step
chunks (N=512 matmuls) double-buffered against the recurrence. The head
matmul runs on host in fp32.
"""
import numpy as np

B, T, F, H = 256, 512, 64, 512
NCORES = 8
BC = B // NCORES          # 32 batch per core
NM = 12                   # 3H/128 gate-row chunks (0-3 r, 4-7 z, 8-11 n)
NK = 4                    # H/128 state chunks
TCHUNK = 16               # steps per gi prefetch chunk

_cache = {}


def _build(Tsteps):
    import concourse.bass as bass
    import concourse.mybir as mybir
    from concourse.tile import TileContext
    from concourse.vector_clock import ScopedClock
    from bass_rust import SyncInfo

    MAXW = 1  # walrus TPB sync-wait slots per instruction

    class TC(TileContext):
        # walrus rejects >MAXW sync waits on one instruction; hoist the excess
        # onto same-engine NOPs inserted right before the offender.
        def _split_waits(self):
            nc = self.nc
            cur = nc.cur_bb.bb
            for fn in nc.m.functions:
                for bb in fn.blocks:
                    insts = bb.instructions
                    if not any(
                        i.sync_info and len(i.sync_info.on_wait) > MAXW
                        for i in insts
                    ):
                        continue
                    new_l = []
                    for inst in insts:
                        si = inst.sync_info
                        w = list(si.on_wait) if si else []
                        if len(w) > MAXW:
                            keep, excess = w[:MAXW], w[MAXW:]
                            for j in range(0, len(excess), MAXW):
                                nop = nc.engines[inst.engine].nop().ins
                                assert cur.instructions.pop() is nop
                                nop.sync_info = SyncInfo(
                                    on_wait=excess[j:j + MAXW], on_update=[])
                                new_l.append(nop)
                            inst.sync_info = SyncInfo(
                                on_wait=keep, on_update=list(si.on_update))
                        new_l.append(inst)
                    bb.instructions[:] = new_l

        def _drain_and_barrier(self, tick_clock, wait_clock):
            drain_inst = self.nc.sync.drain()
            wait_clock.add_sem_waits(
                drain_inst.ins, ScopedClock({None: tick_clock.global_clock})
            )
            self._split_waits()
            self.nc.all_engine_barrier()
            popped = self.nc._tile_sem_poison_stack.pop()
            assert popped is self._sem_poison
            self.nc.clear_and_free_semaphores(list(self.sems.allocated().values()))
            self.nc.all_engine_barrier()

    dt = mybir.dt
    AF = mybir.ActivationFunctionType
    nchunks = Tsteps // TCHUNK
    nc = bass.Bass("TRN2", target_bir_lowering=False, debug=False,
                   num_devices=NCORES)

    xT = nc.declare_dram_parameter("xT", [F, Tsteps * BC], dt.bfloat16, isOutput=False)
    Whh = nc.declare_dram_parameter("Whh", [128, NM * NK * 128], dt.bfloat16, isOutput=False)
    Wih = nc.declare_dram_parameter("Wih", [F, NM * 128], dt.bfloat16, isOutput=False)
    biasv = nc.declare_dram_parameter("biasv", [128, NM], dt.float32, isOutput=False)
    Bn = nc.declare_dram_parameter("Bn", [128, NK * BC], dt.float32, isOutput=False)
    hout = nc.declare_dram_parameter("hout", [128, NK * BC], dt.bfloat16, isOutput=True)

    with TC(nc) as tc:
        with (
            tc.tile_pool(name="const", bufs=1) as constp,
            tc.tile_pool(name="gi", bufs=2) as gip,
            tc.tile_pool(name="pgi", bufs=2, space="PSUM") as pgip,
            tc.tile_pool(name="prz", bufs=2, space="PSUM") as przp,
            tc.tile_pool(name="pn", bufs=2, space="PSUM") as pnp,
            tc.tile_pool(name="ew", bufs=3) as ewp,
        ):
            whh_sb = constp.tile([128, NM * NK * 128], dt.bfloat16, tag="whh")
            wih_sb = constp.tile([F, NM * 128], dt.bfloat16, tag="wih")
            xt_sb = constp.tile([F, Tsteps * BC], dt.bfloat16, tag="xt")
            bias_sb = constp.tile([128, NM], dt.float32, tag="bias")
            bn_sb = constp.tile([128, NK * BC], dt.float32, tag="bn")
            h_bf = constp.tile([128, NK * BC], dt.bfloat16, tag="h")

            nc.sync.dma_start(out=whh_sb[:], in_=Whh[:])
            nc.sync.dma_start(out=wih_sb[:], in_=Wih[:])
            nc.sync.dma_start(out=xt_sb[:], in_=xT[:])
            nc.sync.dma_start(out=bias_sb[:], in_=biasv[:])
            nc.sync.dma_start(out=bn_sb[:], in_=Bn[:])
            nc.gpsimd.memset(h_bf[:], 0.0)

            # gi buffers: [128, NM, TCHUNK*BC] (chunk m stride TCHUNK*BC)
            CW = TCHUNK * BC  # 512

            def gi_mm(gi_tile, c, m):
                pg = pgip.tile([128, CW], dt.float32, tag="pgi")
                nc.tensor.matmul(pg[:], wih_sb[:, m * 128:(m + 1) * 128],
                                 xt_sb[:, c * CW:(c + 1) * CW],
                                 start=True, stop=True)
                nc.scalar.activation(gi_tile[:, m, :], pg[:], AF.Identity,
                                     bias=bias_sb[:, m:m + 1])

            gi_tiles = []
            g0 = gip.tile([128, NM, CW], dt.float32, tag="gi")
            for m in range(NM):
                gi_mm(g0, 0, m)
            gi_tiles.append(g0)

            for c in range(nchunks):
                gcur = gi_tiles[c]
                gnxt = None
                if c + 1 < nchunks:
                    gnxt = gip.tile([128, NM, CW], dt.float32, tag="gi")
                    gi_tiles.append(gnxt)
                for s in range(TCHUNK):
                    sl = slice(s * BC, (s + 1) * BC)
                    prz = przp.tile([128, 8 * BC], dt.float32, tag="prz")
                    pn = pnp.tile([128, NK * BC], dt.float32, tag="pn")
                    for m in range(8):
                        for k in range(NK):
                            nc.tensor.matmul(
                                prz[:, m * BC:(m + 1) * BC],
                                whh_sb[:, (m * NK + k) * 128:(m * NK + k + 1) * 128],
                                h_bf[:, k * BC:(k + 1) * BC],
                                start=(k == 0), stop=(k == NK - 1))
                    for m in range(8, NM):
                        for k in range(NK):
                            nc.tensor.matmul(
                                pn[:, (m - 8) * BC:(m - 7) * BC],
                                whh_sb[:, (m * NK + k) * 128:(m * NK + k + 1) * 128],
                                h_bf[:, k * BC:(k + 1) * BC],
                                start=(k == 0), stop=(k == NK - 1))
                    # prefetch next chunk's gi (spread over steps)
                    if gnxt is not None and s < NM:
                        gi_mm(gnxt, c + 1, s)
                    # elementwise
                    rz = ewp.tile([128, 8 * BC], dt.float32, tag="rz")
                    nc.vector.tensor_add(rz[:], prz[:], gcur[:, 0:8, sl])
                    sig = ewp.tile([128, 8 * BC], dt.float32, tag="sig")
                    nc.scalar.activation(sig[:], rz[:], AF.Sigmoid)
                    t1 = ewp.tile([128, NK * BC], dt.float32, tag="t1")
                    nc.vector.tensor_add(t1[:], pn[:], bn_sb[:])
                    t2 = ewp.tile([128, NK * BC], dt.float32, tag="t2")
                    nc.vector.tensor_mul(t2[:], sig[:, 0:NK * BC], t1[:])
                    t3 = ewp.tile([128, NK * BC], dt.float32, tag="t3")
                    nc.vector.tensor_add(t3[:], t2[:], gcur[:, 8:NM, sl])
                    nt = ewp.tile([128, NK * BC], dt.float32, tag="nt")
                    nc.scalar.activation(nt[:], t3[:], AF.Tanh)
                    for k in range(NK):
                        ks = slice(k * BC, (k + 1) * BC)
                        zs = slice(NK * BC + k * BC, NK * BC + (k + 1) * BC)
                        dk = ewp.tile([128, BC], dt.float32, tag="d")
                        nc.vector.tensor_sub(dk[:], h_bf[:, ks], nt[:, ks])
                        ek = ewp.tile([128, BC], dt.float32, tag="e")
                        nc.vector.tensor_mul(ek[:], sig[:, zs], dk[:])
                        nc.vector.tensor_add(h_bf[:, ks], nt[:, ks], ek[:])

            nc.sync.dma_start(out=hout[:], in_=h_bf[:])
    return nc


def kernel(x, W_ih, W_hh, b_ih, b_hh, head_w, head_b):
    import ml_dtypes
    from concourse.bass_utils import run_bass_kernel_spmd

    Tsteps = x.shape[1]
    if Tsteps not in _cache:
        _cache[Tsteps] = _build(Tsteps)
    nc = _cache[Tsteps]

    bf16 = ml_dtypes.bfloat16
    # weights, shared across cores
    whh = np.ascontiguousarray(
        np.transpose(W_hh.reshape(NM, 128, NK, 128), (3, 0, 2, 1))
    ).reshape(128, NM * NK * 128).astype(bf16)
    wih = np.ascontiguousarray(W_ih.T).astype(bf16)
    ball = b_ih + b_hh
    biasv = np.empty((128, NM), np.float32)
    for m in range(NM):
        src = ball if m < 8 else b_ih
        biasv[:, m] = src[m * 128:(m + 1) * 128]
    bn = np.ascontiguousarray(
        b_hh[2 * H:3 * H].reshape(NK, 128).T
    )  # [p, k]
    bn_t = np.broadcast_to(bn[:, :, None], (128, NK, BC)).reshape(128, NK * BC)
    bn_t = np.ascontiguousarray(bn_t).astype(np.float32)

    in_maps = []
    for ci in range(NCORES):
        xs = x[ci * BC:(ci + 1) * BC]               # [BC, T, F]
        xt = np.ascontiguousarray(np.transpose(xs, (2, 1, 0))) \
            .reshape(F, Tsteps * BC).astype(bf16)
        in_maps.append({"xT": xt, "Whh": whh, "Wih": wih,
                        "biasv": biasv, "Bn": bn_t})

    res = run_bass_kernel_spmd(nc, in_maps, list(range(NCORES)))
    kernel.last_results = res
    kernel.last_in_maps = in_maps

    h_full = np.empty((B, H), np.float32)
    for ci in range(NCORES):
        hl = np.asarray(res.results[ci]["hout"], np.float32)  # [p, k*BC]
        hl = hl.reshape(128, NK, BC)
        h_full[ci * BC:(ci + 1) * BC] = np.transpose(hl, (2, 1, 0)).reshape(BC, H)

    y = h_full @ head_w.T.astype(np.float32) + head_b
    return y.squeeze(-1).astype(np.float32)
